# revision 16
# baseline (speedup 1.0000x reference)
"""Trainium2 Bass kernel for LowRankMultiheadAttention (B=32, S=400, E=1024, H=16).

Strategy: data-parallel over batch across 8 NeuronCores (4 batches/core).
Per core, activations live in "transposed space" (embed dim on partitions) so
every matmul contracts over the SBUF partition dim:

  posT   = WpT.T @ peT                      (positional projection, once)
  qT     = WqT.T @ xqT + (posT + bp + bq)   (per batch, [E, S] layout)
  kT     = WkT.T @ xkT + (posT + bp + bk)
  v_nat  = xvT.T @ WvT + (pos_nat+bp+bv)    (natural [S, E] layout, 65-stride
                                             per head with a ones column)
  per head pair (2 heads share a 128-partition chunk):
    scoresT    = kT_head.T @ qT_head        (row-packed, 2 heads concurrent)
    expT       = Exp(0.125 * scoresT)       (2 key-chunks fused per ACT op)
    attnv      = v_aug.T @ expT             (accum over 4 key chunks; row 64 =
                                             softmax denominator via ones col)
    out_head   = attnv[0:64] * bcast(approx_recip(attnv[64]))
  outT   = WoT.T @ attn_outT; final = gates*outT + gates*bo

Emission interleaves batch b's attention with batch b+1's projections so the
TensorE never idles long enough for the HAM clock-gate to re-throttle.
Host side does layout prep only (transpose/cast/shard + final gather).
"""

from contextlib import ExitStack

import numpy as np
import ml_dtypes

B, S, E, H, D = 32, 400, 1024, 16, 64
NC = 8
BPC = B // NC          # batches per core
SP = 512               # padded key length
CH = E // 128          # 128-partition chunks of the embed dim
SCALE = 1.0 / 8.0      # 1/sqrt(D)
P = 128

_cache: dict = {}
_in_maps_cache = None


def _build(use_mask: bool):
    import concourse.mybir as mybir
    import concourse.tile as tile
    from concourse import bacc

    F32 = mybir.dt.float32
    BF16 = mybir.dt.bfloat16
    AF = mybir.ActivationFunctionType

    nc = bacc.Bacc("TRN2", target_bir_lowering=False, debug=False,
                   enable_asserts=False)

    xq_d = nc.dram_tensor("xq", [BPC, E, S], BF16, kind="ExternalInput")
    xk_d = nc.dram_tensor("xk", [BPC, E, S], BF16, kind="ExternalInput")
    xv_d = nc.dram_tensor("xv", [BPC, E, S], BF16, kind="ExternalInput")
    wq_d = nc.dram_tensor("wqT", [E, E], BF16, kind="ExternalInput")
    wk_d = nc.dram_tensor("wkT", [E, E], BF16, kind="ExternalInput")
    wv_d = nc.dram_tensor("wvT", [E, E], BF16, kind="ExternalInput")
    wo_d = nc.dram_tensor("woT", [E, E], BF16, kind="ExternalInput")
    wp_d = nc.dram_tensor("wpT", [E, E], BF16, kind="ExternalInput")
    pe_d = nc.dram_tensor("peT", [E, S], BF16, kind="ExternalInput")
    bpq_d = nc.dram_tensor("bpq", [P, CH], F32, kind="ExternalInput")
    bpk_d = nc.dram_tensor("bpk", [P, CH], F32, kind="ExternalInput")
    bvp_d = nc.dram_tensor("bvp", [1, E], BF16, kind="ExternalInput")
    bo_d = nc.dram_tensor("bo_col", [P, CH], F32, kind="ExternalInput")
    g_d = nc.dram_tensor("gates11", [1, 1], F32, kind="ExternalInput")
    if use_mask:
        mask_d = nc.dram_tensor("maskT", [SP, S], BF16, kind="ExternalInput")
    out_d = nc.dram_tensor("outT", [BPC, E, S], F32, kind="ExternalOutput")

    def dram3(d):  # [E, x] row-major -> [128, CH, x] partition view
        return d.ap().rearrange("(c p) t -> p c t", p=P)

    with tile.TileContext(nc) as tc, ExitStack() as ctx:
        consts = ctx.enter_context(tc.tile_pool(name="consts", bufs=1))
        wpool = ctx.enter_context(tc.tile_pool(name="weights", bufs=1))

        PQ = consts.tile([P, CH, S], BF16, tag="PQ")
        PK = consts.tile([P, CH, S], BF16, tag="PK")
        PV = consts.tile([P, 4, E], BF16, tag="PV")
        bvpb = consts.tile([P, E], BF16, tag="bvpb")
        ones_bf = consts.tile([1, P], BF16, tag="ones_bf")
        ones_f32 = consts.tile([1, P], F32, tag="ones_f32")
        zeros_col = consts.tile([P, 1], F32, tag="zeros_col")
        gates_col = consts.tile([P, 1], F32, tag="gates_col")
        bo_g = consts.tile([P, CH], F32, tag="bo_g")
        bpq = consts.tile([P, CH], F32, tag="bpq")
        bpk = consts.tile([P, CH], F32, tag="bpk")
        bo_c = consts.tile([P, CH], F32, tag="bo_c")
        if use_mask:
            maskT = consts.tile([P, 4, S], BF16, tag="maskT")
            nc.sync.dma_start(maskT[:], mask_d.ap().rearrange("(c p) t -> p c t", p=P))

        nc.gpsimd.dma_start(bpq[:], bpq_d.ap())
        nc.gpsimd.dma_start(bpk[:], bpk_d.ap())
        nc.gpsimd.dma_start(bo_c[:], bo_d.ap())
        nc.vector.memset(ones_bf[:], 1.0)
        nc.vector.memset(ones_f32[:], 1.0)
        nc.vector.memset(zeros_col[:], 0.0)

        # ---------------- startup: positional projections + scalar prep ----
        with tc.tile_pool(name="startup", bufs=1) as spool, \
             tc.tile_pool(name="spsum", bufs=2, space="PSUM") as spsum:
            wpT = spool.tile([P, CH, E], BF16, tag="wpT")
            peT = spool.tile([P, CH, S], BF16, tag="peT")
            g_sb = spool.tile([1, 1], F32, tag="g_sb")
            bvp_sb = spool.tile([1, E], BF16, tag="bvp_sb")
            nc.sync.dma_start(peT[:], dram3(pe_d))
            nc.sync.dma_start(wpT[:], dram3(wp_d))
            nc.gpsimd.dma_start(g_sb[:], g_d.ap())
            nc.gpsimd.dma_start(bvp_sb[:], bvp_d.ap())

            # gates broadcast down partitions (K=1 fp32 matmul)
            gp = spsum.tile([P, 512], F32, tag="sp")
            nc.tensor.matmul(gp[:, 0:1], ones_f32[:], g_sb[:], start=True, stop=True)
            nc.scalar.copy(gates_col[:], gp[:, 0:1])
            nc.vector.tensor_scalar_mul(bo_g[:], bo_c[:], gates_col[:, 0:1])

            # (bp+bv) broadcast down partitions
            for hf in range(2):
                ps = spsum.tile([P, 512], F32, tag="sp")
                nc.tensor.matmul(ps[:], ones_bf[:], bvp_sb[0:1, hf * 512:(hf + 1) * 512],
                                 start=True, stop=True)
                nc.scalar.copy(bvpb[:, hf * 512:(hf + 1) * 512], ps[:])

            # posT -> PQ / PK  (transposed orientation)
            for ec in range(CH):
                ps = spsum.tile([P, 512], F32, tag="sp")
                for dc in range(CH):
                    nc.tensor.matmul(ps[:, 0:S], wpT[:, dc, ec * P:(ec + 1) * P],
                                     peT[:, dc, :], start=dc == 0, stop=dc == CH - 1)
                nc.scalar.activation(PQ[:, ec, :], ps[:, 0:S], AF.Identity,
                                     bias=bpq[:, ec:ec + 1])
                nc.scalar.activation(PK[:, ec, :], ps[:, 0:S], AF.Identity,
                                     bias=bpk[:, ec:ec + 1])

            # pos_nat -> PV  (natural orientation)
            for tt in range(4):
                tsz = P if tt < 3 else S - 3 * P
                for hf in range(2):
                    ps = spsum.tile([P, 512], F32, tag="sp")
                    for dc in range(CH):
                        nc.tensor.matmul(ps[0:tsz, :], peT[:, dc, tt * P:tt * P + tsz],
                                         wpT[:, dc, hf * 512:(hf + 1) * 512],
                                         start=dc == 0, stop=dc == CH - 1)
                    nc.vector.tensor_add(PV[0:tsz, tt, hf * 512:(hf + 1) * 512],
                                         ps[0:tsz, :],
                                         bvpb[0:tsz, hf * 512:(hf + 1) * 512])

        # ---------------- resident weights --------------------------------
        wq = wpool.tile([P, CH, E], BF16, tag="wq")
        wk = wpool.tile([P, CH, E], BF16, tag="wk")
        wv = wpool.tile([P, CH, E], BF16, tag="wv")
        wo = wpool.tile([P, CH, E], BF16, tag="wo")
        nc.gpsimd.dma_start(wq[:], dram3(wq_d))
        nc.gpsimd.dma_start(wk[:], dram3(wk_d))
        nc.gpsimd.dma_start(wv[:], dram3(wv_d))
        nc.gpsimd.dma_start(wo[:], dram3(wo_d))

        # ---------------- main pools ---------------------------------------
        xin = ctx.enter_context(tc.tile_pool(name="xin", bufs=3))
        qtp = ctx.enter_context(tc.tile_pool(name="qt", bufs=2))
        ktp = ctx.enter_context(tc.tile_pool(name="kt", bufs=2))
        vap = ctx.enter_context(tc.tile_pool(name="va", bufs=2))
        aop = ctx.enter_context(tc.tile_pool(name="ao", bufs=3))
        exq = ctx.enter_context(tc.tile_pool(name="ex", bufs=4))
        bcp = ctx.enter_context(tc.tile_pool(name="bc", bufs=3))
        rcp = ctx.enter_context(tc.tile_pool(name="rc", bufs=4))
        fnp = ctx.enter_context(tc.tile_pool(name="fn", bufs=3))
        mmp = ctx.enter_context(tc.tile_pool(name="mm", bufs=2, space="PSUM"))
        scp = ctx.enter_context(tc.tile_pool(name="sc", bufs=2, space="PSUM"))
        avp = ctx.enter_context(tc.tile_pool(name="av", bufs=2, space="PSUM"))

        tiles = {}

        def load_batch(b):
            t = {}
            t["xq"] = xin.tile([P, CH, S], BF16, tag="xin", name=f"xq{b}")
            nc.sync.dma_start(t["xq"][:], xq_d.ap()[b].rearrange("(c p) t -> p c t", p=P))
            t["xk"] = xin.tile([P, CH, S], BF16, tag="xin", name=f"xk{b}")
            nc.sync.dma_start(t["xk"][:], xk_d.ap()[b].rearrange("(c p) t -> p c t", p=P))
            t["xv"] = xin.tile([P, CH, S], BF16, tag="xin", name=f"xv{b}")
            nc.sync.dma_start(t["xv"][:], xv_d.ap()[b].rearrange("(c p) t -> p c t", p=P))
            t["qT"] = qtp.tile([P, CH, S], BF16, tag="qt", name=f"qT{b}")
            t["kT"] = ktp.tile([P, CH, SP], BF16, tag="kt", name=f"kT{b}")
            nc.vector.memset(t["kT"][:, :, S:SP], 0.0)
            va = vap.tile([P, 4, H, D + 1], BF16, tag="va", name=f"va{b}")
            nc.vector.memset(va[:, :, :, D], 1.0)   # ones column
            nc.vector.memset(va[:, 3, :, :], 0.0)   # zero padded key rows
            nc.vector.memset(va[0:S - 3 * P, 3, :, D], 1.0)  # restore valid ones
            t["va"] = va
            t["ao"] = aop.tile([P, CH, S], BF16, tag="ao", name=f"ao{b}")
            tiles[b] = t

        def proj_q_chunk(b, ec):
            t = tiles[b]
            ps = mmp.tile([P, 512], F32, tag="mm", name=f"psq{b}_{ec}")
            for dc in range(CH):
                nc.tensor.matmul(ps[:, 0:S], wq[:, dc, ec * P:(ec + 1) * P],
                                 t["xq"][:, dc, :], start=dc == 0, stop=dc == CH - 1)
            nc.vector.tensor_add(t["qT"][:, ec, :], ps[:, 0:S], PQ[:, ec, :])

        def proj_k_chunk(b, ec):
            t = tiles[b]
            ps = mmp.tile([P, 512], F32, tag="mm", name=f"psk{b}_{ec}")
            for dc in range(CH):
                nc.tensor.matmul(ps[:, 0:S], wk[:, dc, ec * P:(ec + 1) * P],
                                 t["xk"][:, dc, :], start=dc == 0, stop=dc == CH - 1)
            nc.vector.tensor_add(t["kT"][:, ec, 0:S], ps[:, 0:S], PK[:, ec, :])

        def proj_v_chunk(b, u):
            t = tiles[b]
            tt, hf = u // 2, u % 2
            tsz = P if tt < 3 else S - 3 * P
            ps = mmp.tile([P, 512], F32, tag="mm", name=f"psv{b}_{u}")
            for dc in range(CH):
                nc.tensor.matmul(ps[0:tsz, :], t["xv"][:, dc, tt * P:tt * P + tsz],
                                 wv[:, dc, hf * 512:(hf + 1) * 512],
                                 start=dc == 0, stop=dc == CH - 1)
            nc.vector.tensor_add(
                t["va"][0:tsz, tt, hf * 8:(hf + 1) * 8, 0:D],
                ps[0:tsz, :].rearrange("p (h d) -> p h d", d=D),
                PV[0:tsz, tt, hf * 512:(hf + 1) * 512].rearrange(
                    "p (h d) -> p h d", d=D))

        def attn_pair(b, o, pull=None):
            t = tiles[b]
            qT, kT, va, ao = t["qT"], t["kT"], t["va"], t["ao"]
            av0 = avp.tile([D + 1, S], F32, tag="av", name=f"av0_{b}_{o}")
            av1 = avp.tile([D + 1, S], F32, tag="av", name=f"av1_{b}_{o}")
            for cc in range(2):
                sA = scp.tile([P, 2, 512], F32, tag="sc", name=f"sA{b}_{o}_{cc}")
                sB = scp.tile([P, 2, 512], F32, tag="sc", name=f"sB{b}_{o}_{cc}")
                for i in range(2):
                    c = 2 * cc + i
                    nc.tensor.matmul(sA[:, i, 0:S], kT[0:D, o, c * P:(c + 1) * P],
                                     qT[0:D, o, :], start=True, stop=True,
                                     tile_position=(0, 0))
                    nc.tensor.matmul(sB[:, i, 0:S], kT[D:P, o, c * P:(c + 1) * P],
                                     qT[D:P, o, :], start=True, stop=True,
                                     tile_position=(64, 0))
                    if use_mask:
                        nc.vector.tensor_add(sA[:, i, 0:S], sA[:, i, 0:S],
                                             maskT[:, c, :])
                        nc.vector.tensor_add(sB[:, i, 0:S], sB[:, i, 0:S],
                                             maskT[:, c, :])
                eA = exq.tile([P, 2, S], BF16, tag="ex", name=f"eA{b}_{o}_{cc}")
                eB = exq.tile([P, 2, S], BF16, tag="ex", name=f"eB{b}_{o}_{cc}")
                nc.scalar.activation(eA[:], sA[:, :, 0:S], AF.Exp, bias=zeros_col[:],
                                     scale=SCALE)
                nc.scalar.activation(eB[:], sB[:, :, 0:S], AF.Exp, bias=zeros_col[:],
                                     scale=SCALE)
                for i in range(2):
                    c = 2 * cc + i
                    nc.tensor.matmul(av0[:], va[:, c, 2 * o, :], eA[:, i, :],
                                     start=c == 0, stop=c == 3)
                    nc.tensor.matmul(av1[:], va[:, c, 2 * o + 1, :], eB[:, i, :],
                                     start=c == 0, stop=c == 3)
                if pull is not None:
                    pull(1)
            d0 = rcp.tile([1, S], F32, tag="den", name=f"d0{b}_{o}")
            d1 = rcp.tile([1, S], F32, tag="den", name=f"d1{b}_{o}")
            nc.vector.tensor_copy(d0[:], av0[D:D + 1, :])
            nc.vector.tensor_copy(d1[:], av1[D:D + 1, :])
            r0f = rcp.tile([1, S], F32, tag="rcf", name=f"r0f{b}_{o}")
            r1f = rcp.tile([1, S], F32, tag="rcf", name=f"r1f{b}_{o}")
            nc.vector.reciprocal_approx_fast(r0f[:], d0[:])
            nc.vector.reciprocal_approx_fast(r1f[:], d1[:])
            bc0 = bcp.tile([D, S], F32, tag="bc0", name=f"bc0{b}_{o}")
            bc1 = bcp.tile([D, S], F32, tag="bc1", name=f"bc1{b}_{o}")
            nc.gpsimd.partition_broadcast(bc0[:], r0f[:])
            nc.gpsimd.partition_broadcast(bc1[:], r1f[:])
            nc.vector.tensor_mul(ao[0:D, o, :], av0[0:D, :], bc0[:])
            nc.vector.tensor_mul(ao[D:P, o, :], av1[0:D, :], bc1[:])

        def outproj_chunk(b, oc):
            t = tiles[b]
            ps = mmp.tile([P, 512], F32, tag="mm", name=f"pso{b}_{oc}")
            for ec in range(CH):
                nc.tensor.matmul(ps[:, 0:S], wo[:, ec, oc * P:(oc + 1) * P],
                                 t["ao"][:, ec, :], start=ec == 0,
                                 stop=ec == CH - 1)
            fn = fnp.tile([P, S], F32, tag="fn", name=f"fn{b}_{oc}")
            nc.scalar.activation(fn[:], ps[:, 0:S], AF.Identity,
                                 bias=bo_g[:, oc:oc + 1],
                                 scale=gates_col[:, 0:1])
            nc.sync.dma_start(
                out_d.ap()[b].rearrange("(c p) t -> p c t", p=P)[:, oc, :],
                fn[:])

        def proj_units(b):
            return ([lambda ec=ec: proj_q_chunk(b, ec) for ec in range(CH)]
                    + [lambda ec=ec: proj_k_chunk(b, ec) for ec in range(CH)]
                    + [lambda u=u: proj_v_chunk(b, u) for u in range(8)])

        # prologue: batch 0 projections
        load_batch(0)
        for u in proj_units(0):
            u()

        for b in range(BPC):
            units = []
            if b + 1 < BPC:
                load_batch(b + 1)
                units = proj_units(b + 1)
            if b >= 1:
                units = units + [lambda oc=oc, bb=b - 1: outproj_chunk(bb, oc)
                                 for oc in range(CH)]
            # attention(b) interleaved with projections(b+1) + outproj(b-1)
            it = iter(units)

            def pull(n):
                for _ in range(n):
                    u = next(it, None)
                    if u is not None:
                        u()

            npair = (len(units) + CH - 1) // CH if units else 0
            for o in range(CH):
                attn_pair(b, o, pull)
                pull(max(npair - 2, 0))
            pull(len(units))
            if b >= 1:
                del tiles[b - 1]
        for oc in range(CH):
            outproj_chunk(BPC - 1, oc)
        del tiles[BPC - 1]

    nc.compile()
    return nc


def _col(v):
    """[E] bias -> [128, CH] per-partition column layout (e = c*128 + p)."""
    return np.ascontiguousarray(v.reshape(CH, P).T.astype(np.float32))


def kernel(query, key, value, attn_mask, pe, Wq, bq, Wk, bk, Wv, bv, Wp, bp,
           Wo, bo, gates):
    bf16 = ml_dtypes.bfloat16
    f32 = np.float32
    query, key, value = (np.asarray(x, f32) for x in (query, key, value))
    attn_mask = np.asarray(attn_mask, f32)
    pe = np.asarray(pe, f32)
    Wq, bq, Wk, bk, Wv, bv, Wp, bp, Wo, bo, gates = (
        np.asarray(x, f32) for x in (Wq, bq, Wk, bk, Wv, bv, Wp, bp, Wo, bo,
                                     gates))

    use_mask = bool(np.any(attn_mask))
    key_nc = ("nc", use_mask)
    if key_nc not in _cache:
        _cache[key_nc] = _build(use_mask)
    nc = _cache[key_nc]

    def t_bf16(x):  # [.., A, B] -> [.., B, A] contiguous bf16
        return np.ascontiguousarray(np.swapaxes(x, -1, -2)).astype(bf16)

    common = {
        "wqT": t_bf16(Wq), "wkT": t_bf16(Wk), "wvT": t_bf16(Wv),
        "woT": t_bf16(Wo), "wpT": t_bf16(Wp),
        "peT": t_bf16(pe[0, :S]),
        "bpq": _col(bp + bq), "bpk": _col(bp + bk),
        "bvp": np.ascontiguousarray((bp + bv)[None, :]).astype(bf16),
        "bo_col": _col(bo),
        "gates11": gates.reshape(1, 1).astype(f32),
    }
    if use_mask:
        mT = np.zeros((SP, S), f32)
        mT[:S, :] = attn_mask.T
        common["maskT"] = mT.astype(bf16)

    qT = t_bf16(query)   # [B, E, S]
    kT = t_bf16(key)
    vT = t_bf16(value)

    in_maps = []
    for i in range(NC):
        sl = slice(i * BPC, (i + 1) * BPC)
        m = dict(common)
        m["xq"] = np.ascontiguousarray(qT[sl])
        m["xk"] = np.ascontiguousarray(kT[sl])
        m["xv"] = np.ascontiguousarray(vT[sl])
        in_maps.append(m)

    global _in_maps_cache
    _in_maps_cache = in_maps

    from concourse.bass_utils import run_bass_kernel_spmd
    res = run_bass_kernel_spmd(nc, in_maps, core_ids=list(range(NC)))

    out = np.empty((B, S, E), f32)
    for i in range(NC):
        outT = res.results[i]["outT"]          # [BPC, E, S]
        out[i * BPC:(i + 1) * BPC] = np.swapaxes(outT, 1, 2)
    return out


# revision 19
# speedup vs baseline: 1.0300x; 1.0300x over previous
"""Trainium2 Bass kernel for LowRankMultiheadAttention (B=32, S=400, E=1024, H=16).

Strategy: data-parallel over batch across 8 NeuronCores (4 batches/core).
Per core, activations live in "transposed space" (embed dim on partitions) so
every matmul contracts over the SBUF partition dim:

  posT   = WpT.T @ peT                      (positional projection, once)
  qT     = WqT.T @ xqT + (posT + bp + bq)   (per batch, [E, S] layout)
  kT     = WkT.T @ xkT + (posT + bp + bk)
  v_nat  = xvT.T @ WvT + (pos_nat+bp+bv)    (natural [S, E] layout, 65-stride
                                             per head with a ones column)
  per head pair (2 heads share a 128-partition chunk):
    scoresT    = kT_head.T @ qT_head        (row-packed, 2 heads concurrent)
    expT       = Exp(0.125 * scoresT)       (2 key-chunks fused per ACT op)
    attnv      = v_aug.T @ expT             (accum over 4 key chunks; row 64 =
                                             softmax denominator via ones col)
    out_head   = attnv[0:64] * bcast(approx_recip(attnv[64]))
  outT   = WoT.T @ attn_outT; final = gates*outT + gates*bo

Emission interleaves batch b's attention with batch b+1's projections so the
TensorE never idles long enough for the HAM clock-gate to re-throttle.
Host side does layout prep only (transpose/cast/shard + final gather).
"""

from contextlib import ExitStack

import numpy as np
import ml_dtypes

B, S, E, H, D = 32, 400, 1024, 16, 64
NC = 8
BPC = B // NC          # batches per core
SP = 512               # padded key length
CH = E // 128          # 128-partition chunks of the embed dim
SCALE = 1.0 / 8.0      # 1/sqrt(D)
P = 128

_cache: dict = {}
_in_maps_cache = None


def _build(use_mask: bool):
    import concourse.mybir as mybir
    import concourse.tile as tile
    from concourse import bacc

    F32 = mybir.dt.float32
    BF16 = mybir.dt.bfloat16
    AF = mybir.ActivationFunctionType

    nc = bacc.Bacc("TRN2", target_bir_lowering=False, debug=False,
                   enable_asserts=False)

    xq_d = nc.dram_tensor("xq", [BPC, E, S], BF16, kind="ExternalInput")
    xk_d = nc.dram_tensor("xk", [BPC, E, S], BF16, kind="ExternalInput")
    xv_d = nc.dram_tensor("xv", [BPC, E, S], BF16, kind="ExternalInput")
    wq_d = nc.dram_tensor("wqT", [E, E], BF16, kind="ExternalInput")
    wk_d = nc.dram_tensor("wkT", [E, E], BF16, kind="ExternalInput")
    wv_d = nc.dram_tensor("wvT", [E, E], BF16, kind="ExternalInput")
    wo_d = nc.dram_tensor("woT", [E, E], BF16, kind="ExternalInput")
    wp_d = nc.dram_tensor("wpT", [E, E], BF16, kind="ExternalInput")
    pe_d = nc.dram_tensor("peT", [E, S], BF16, kind="ExternalInput")
    bpq_d = nc.dram_tensor("bpq", [P, CH], F32, kind="ExternalInput")
    bpk_d = nc.dram_tensor("bpk", [P, CH], F32, kind="ExternalInput")
    bvp_d = nc.dram_tensor("bvp", [1, E], BF16, kind="ExternalInput")
    bo_d = nc.dram_tensor("bo_col", [P, CH], F32, kind="ExternalInput")
    g_d = nc.dram_tensor("gates11", [1, 1], F32, kind="ExternalInput")
    if use_mask:
        mask_d = nc.dram_tensor("maskT", [SP, S], BF16, kind="ExternalInput")
    out_d = nc.dram_tensor("outT", [BPC, E, S], F32, kind="ExternalOutput")

    def dram3(d):  # [E, x] row-major -> [128, CH, x] partition view
        return d.ap().rearrange("(c p) t -> p c t", p=P)

    with tile.TileContext(nc) as tc, ExitStack() as ctx:
        consts = ctx.enter_context(tc.tile_pool(name="consts", bufs=1))
        wpool = ctx.enter_context(tc.tile_pool(name="weights", bufs=1))

        PQ = consts.tile([P, CH, S], BF16, tag="PQ")
        PK = consts.tile([P, CH, S], BF16, tag="PK")
        PV = consts.tile([P, 4, E], BF16, tag="PV")
        bvpb = consts.tile([P, E], BF16, tag="bvpb")
        ones_bf = consts.tile([1, P], BF16, tag="ones_bf")
        ones_f32 = consts.tile([1, P], F32, tag="ones_f32")
        zeros_col = consts.tile([P, 1], F32, tag="zeros_col")
        gates_col = consts.tile([P, 1], F32, tag="gates_col")
        bo_g = consts.tile([P, CH], F32, tag="bo_g")
        bpq = consts.tile([P, CH], F32, tag="bpq")
        bpk = consts.tile([P, CH], F32, tag="bpk")
        bo_c = consts.tile([P, CH], F32, tag="bo_c")
        if use_mask:
            maskT = consts.tile([P, 4, S], BF16, tag="maskT")
            nc.sync.dma_start(maskT[:], mask_d.ap().rearrange("(c p) t -> p c t", p=P))

        nc.gpsimd.dma_start(bpq[:], bpq_d.ap())
        nc.gpsimd.dma_start(bpk[:], bpk_d.ap())
        nc.gpsimd.dma_start(bo_c[:], bo_d.ap())
        nc.vector.memset(ones_bf[:], 1.0)
        nc.vector.memset(ones_f32[:], 1.0)
        nc.vector.memset(zeros_col[:], 0.0)

        # ---------------- startup: positional projections + scalar prep ----
        with tc.tile_pool(name="startup", bufs=1) as spool, \
             tc.tile_pool(name="spsum", bufs=2, space="PSUM") as spsum:
            wpT = spool.tile([P, CH, E], BF16, tag="wpT")
            peT = spool.tile([P, CH, S], BF16, tag="peT")
            g_sb = spool.tile([1, 1], F32, tag="g_sb")
            bvp_sb = spool.tile([1, E], BF16, tag="bvp_sb")
            nc.sync.dma_start(peT[:], dram3(pe_d))
            nc.sync.dma_start(wpT[:], dram3(wp_d))
            nc.gpsimd.dma_start(g_sb[:], g_d.ap())
            nc.gpsimd.dma_start(bvp_sb[:], bvp_d.ap())

            # gates broadcast down partitions (K=1 fp32 matmul)
            gp = spsum.tile([P, 512], F32, tag="sp")
            nc.tensor.matmul(gp[:, 0:1], ones_f32[:], g_sb[:], start=True, stop=True)
            nc.scalar.copy(gates_col[:], gp[:, 0:1])
            nc.vector.tensor_scalar_mul(bo_g[:], bo_c[:], gates_col[:, 0:1])

            # (bp+bv) broadcast down partitions
            for hf in range(2):
                ps = spsum.tile([P, 512], F32, tag="sp")
                nc.tensor.matmul(ps[:], ones_bf[:], bvp_sb[0:1, hf * 512:(hf + 1) * 512],
                                 start=True, stop=True)
                nc.scalar.copy(bvpb[:, hf * 512:(hf + 1) * 512], ps[:])

            # posT -> PQ / PK  (transposed orientation)
            for ec in range(CH):
                ps = spsum.tile([P, 512], F32, tag="sp")
                for dc in range(CH):
                    nc.tensor.matmul(ps[:, 0:S], wpT[:, dc, ec * P:(ec + 1) * P],
                                     peT[:, dc, :], start=dc == 0, stop=dc == CH - 1)
                nc.scalar.activation(PQ[:, ec, :], ps[:, 0:S], AF.Identity,
                                     bias=bpq[:, ec:ec + 1])
                nc.scalar.activation(PK[:, ec, :], ps[:, 0:S], AF.Identity,
                                     bias=bpk[:, ec:ec + 1])

            # pos_nat -> PV  (natural orientation)
            for tt in range(4):
                tsz = P if tt < 3 else S - 3 * P
                for hf in range(2):
                    ps = spsum.tile([P, 512], F32, tag="sp")
                    for dc in range(CH):
                        nc.tensor.matmul(ps[0:tsz, :], peT[:, dc, tt * P:tt * P + tsz],
                                         wpT[:, dc, hf * 512:(hf + 1) * 512],
                                         start=dc == 0, stop=dc == CH - 1)
                    nc.vector.tensor_add(PV[0:tsz, tt, hf * 512:(hf + 1) * 512],
                                         ps[0:tsz, :],
                                         bvpb[0:tsz, hf * 512:(hf + 1) * 512])

        # ---------------- resident weights --------------------------------
        wq = wpool.tile([P, CH, E], BF16, tag="wq")
        wk = wpool.tile([P, CH, E], BF16, tag="wk")
        wv = wpool.tile([P, CH, E], BF16, tag="wv")
        wo = wpool.tile([P, CH, E], BF16, tag="wo")
        nc.gpsimd.dma_start(wq[:], dram3(wq_d))
        nc.gpsimd.dma_start(wk[:], dram3(wk_d))
        nc.gpsimd.dma_start(wv[:], dram3(wv_d))
        nc.gpsimd.dma_start(wo[:], dram3(wo_d))

        # ---------------- main pools ---------------------------------------
        xin = ctx.enter_context(tc.tile_pool(name="xin", bufs=3))
        qtp = ctx.enter_context(tc.tile_pool(name="qt", bufs=2))
        ktp = ctx.enter_context(tc.tile_pool(name="kt", bufs=2))
        vap = ctx.enter_context(tc.tile_pool(name="va", bufs=2))
        aop = ctx.enter_context(tc.tile_pool(name="ao", bufs=3))
        exq = ctx.enter_context(tc.tile_pool(name="ex", bufs=4))
        bcp = ctx.enter_context(tc.tile_pool(name="bc", bufs=3))
        rcp = ctx.enter_context(tc.tile_pool(name="rc", bufs=4))
        fnp = ctx.enter_context(tc.tile_pool(name="fn", bufs=3))
        mmp = ctx.enter_context(tc.tile_pool(name="mm", bufs=2, space="PSUM"))
        scp = ctx.enter_context(tc.tile_pool(name="sc", bufs=2, space="PSUM"))
        avp = ctx.enter_context(tc.tile_pool(name="av", bufs=2, space="PSUM"))

        tiles = {}

        def load_batch(b):
            t = {}
            t["xq"] = xin.tile([P, CH, S], BF16, tag="xin", name=f"xq{b}")
            nc.sync.dma_start(t["xq"][:], xq_d.ap()[b].rearrange("(c p) t -> p c t", p=P))
            t["xk"] = xin.tile([P, CH, S], BF16, tag="xin", name=f"xk{b}")
            nc.sync.dma_start(t["xk"][:], xk_d.ap()[b].rearrange("(c p) t -> p c t", p=P))
            t["xv"] = xin.tile([P, CH, S], BF16, tag="xin", name=f"xv{b}")
            nc.sync.dma_start(t["xv"][:], xv_d.ap()[b].rearrange("(c p) t -> p c t", p=P))
            t["qT"] = qtp.tile([P, CH, S], BF16, tag="qt", name=f"qT{b}")
            t["kT"] = ktp.tile([P, CH, SP], BF16, tag="kt", name=f"kT{b}")
            nc.vector.memset(t["kT"][:, :, S:SP], 0.0)
            va = vap.tile([P, 4, H, D + 1], BF16, tag="va", name=f"va{b}")
            nc.vector.memset(va[:, :, :, D], 1.0)   # ones column
            nc.vector.memset(va[:, 3, :, :], 0.0)   # zero padded key rows
            nc.vector.memset(va[0:S - 3 * P, 3, :, D], 1.0)  # restore valid ones
            t["va"] = va
            t["ao"] = aop.tile([P, CH, S], BF16, tag="ao", name=f"ao{b}")
            tiles[b] = t

        def proj_q_chunk(b, ec):
            t = tiles[b]
            ps = mmp.tile([P, 512], F32, tag="mm", name=f"psq{b}_{ec}")
            for dc in range(CH):
                nc.tensor.matmul(ps[:, 0:S], wq[:, dc, ec * P:(ec + 1) * P],
                                 t["xq"][:, dc, :], start=dc == 0, stop=dc == CH - 1)
            nc.vector.tensor_add(t["qT"][:, ec, :], ps[:, 0:S], PQ[:, ec, :])

        def proj_k_chunk(b, ec):
            t = tiles[b]
            ps = mmp.tile([P, 512], F32, tag="mm", name=f"psk{b}_{ec}")
            for dc in range(CH):
                nc.tensor.matmul(ps[:, 0:S], wk[:, dc, ec * P:(ec + 1) * P],
                                 t["xk"][:, dc, :], start=dc == 0, stop=dc == CH - 1)
            nc.vector.tensor_add(t["kT"][:, ec, 0:S], ps[:, 0:S], PK[:, ec, :])

        def proj_v_chunk(b, u):
            t = tiles[b]
            tt, hf = u // 2, u % 2
            tsz = P if tt < 3 else S - 3 * P
            ps = mmp.tile([P, 512], F32, tag="mm", name=f"psv{b}_{u}")
            for dc in range(CH):
                nc.tensor.matmul(ps[0:tsz, :], t["xv"][:, dc, tt * P:tt * P + tsz],
                                 wv[:, dc, hf * 512:(hf + 1) * 512],
                                 start=dc == 0, stop=dc == CH - 1)
            nc.vector.tensor_add(
                t["va"][0:tsz, tt, hf * 8:(hf + 1) * 8, 0:D],
                ps[0:tsz, :].rearrange("p (h d) -> p h d", d=D),
                PV[0:tsz, tt, hf * 512:(hf + 1) * 512].rearrange(
                    "p (h d) -> p h d", d=D))

        def attn_pair(b, o, pull=None):
            t = tiles[b]
            qT, kT, va, ao = t["qT"], t["kT"], t["va"], t["ao"]
            av0 = avp.tile([D + 1, S], F32, tag="av", name=f"av0_{b}_{o}")
            av1 = avp.tile([D + 1, S], F32, tag="av", name=f"av1_{b}_{o}")
            for cc in range(2):
                sA = scp.tile([P, 2, 512], F32, tag="sc", name=f"sA{b}_{o}_{cc}")
                sB = scp.tile([P, 2, 512], F32, tag="sc", name=f"sB{b}_{o}_{cc}")
                for i in range(2):
                    c = 2 * cc + i
                    nc.tensor.matmul(sA[:, i, 0:S], kT[0:D, o, c * P:(c + 1) * P],
                                     qT[0:D, o, :], start=True, stop=True,
                                     tile_position=(0, 0))
                    nc.tensor.matmul(sB[:, i, 0:S], kT[D:P, o, c * P:(c + 1) * P],
                                     qT[D:P, o, :], start=True, stop=True,
                                     tile_position=(64, 0))
                    if use_mask:
                        nc.vector.tensor_add(sA[:, i, 0:S], sA[:, i, 0:S],
                                             maskT[:, c, :])
                        nc.vector.tensor_add(sB[:, i, 0:S], sB[:, i, 0:S],
                                             maskT[:, c, :])
                eA = exq.tile([P, 2, S], BF16, tag="ex", name=f"eA{b}_{o}_{cc}")
                eB = exq.tile([P, 2, S], BF16, tag="ex", name=f"eB{b}_{o}_{cc}")
                nc.scalar.activation(eA[:], sA[:, :, 0:S], AF.Exp, bias=zeros_col[:],
                                     scale=SCALE)
                nc.scalar.activation(eB[:], sB[:, :, 0:S], AF.Exp, bias=zeros_col[:],
                                     scale=SCALE)
                for i in range(2):
                    c = 2 * cc + i
                    nc.tensor.matmul(av0[:], va[:, c, 2 * o, :], eA[:, i, :],
                                     start=c == 0, stop=c == 3)
                    nc.tensor.matmul(av1[:], va[:, c, 2 * o + 1, :], eB[:, i, :],
                                     start=c == 0, stop=c == 3)
                if pull is not None:
                    pull(1)
            d0 = rcp.tile([1, S], F32, tag="den", name=f"d0{b}_{o}")
            d1 = rcp.tile([1, S], F32, tag="den", name=f"d1{b}_{o}")
            nc.vector.tensor_copy(d0[:], av0[D:D + 1, :])
            nc.vector.tensor_copy(d1[:], av1[D:D + 1, :])
            r0f = rcp.tile([1, S], F32, tag="rcf", name=f"r0f{b}_{o}")
            r1f = rcp.tile([1, S], F32, tag="rcf", name=f"r1f{b}_{o}")
            nc.vector.reciprocal_approx_fast(r0f[:], d0[:])
            nc.vector.reciprocal_approx_fast(r1f[:], d1[:])
            bc0 = bcp.tile([D, S], F32, tag="bc0", name=f"bc0{b}_{o}")
            bc1 = bcp.tile([D, S], F32, tag="bc1", name=f"bc1{b}_{o}")
            nc.gpsimd.partition_broadcast(bc0[:], r0f[:])
            nc.gpsimd.partition_broadcast(bc1[:], r1f[:])
            nc.vector.tensor_mul(ao[0:D, o, :], av0[0:D, :], bc0[:])
            nc.vector.tensor_mul(ao[D:P, o, :], av1[0:D, :], bc1[:])

        def outproj_chunk(b, oc):
            t = tiles[b]
            ps = mmp.tile([P, 512], F32, tag="mm", name=f"pso{b}_{oc}")
            for ec in range(CH):
                nc.tensor.matmul(ps[:, 0:S], wo[:, ec, oc * P:(oc + 1) * P],
                                 t["ao"][:, ec, :], start=ec == 0,
                                 stop=ec == CH - 1)
            fn = fnp.tile([P, S], F32, tag="fn", name=f"fn{b}_{oc}")
            nc.scalar.activation(fn[:], ps[:, 0:S], AF.Identity,
                                 bias=bo_g[:, oc:oc + 1],
                                 scale=gates_col[:, 0:1])
            nc.sync.dma_start(
                out_d.ap()[b].rearrange("(c p) t -> p c t", p=P)[:, oc, :],
                fn[:])

        def first_half(b):
            # chunks needed by attention pairs 0-3: q/k ec 0-3, v heads 0-7
            return ([lambda ec=ec: proj_q_chunk(b, ec) for ec in range(4)]
                    + [lambda ec=ec: proj_k_chunk(b, ec) for ec in range(4)]
                    + [lambda u=u: proj_v_chunk(b, u) for u in (0, 2, 4, 6)])

        def second_half(b):
            # chunks first needed at pair 4: woven into pairs 0-3 of attn(b)
            return ([lambda ec=ec: proj_q_chunk(b, ec) for ec in range(4, CH)]
                    + [lambda ec=ec: proj_k_chunk(b, ec) for ec in range(4, CH)]
                    + [lambda u=u: proj_v_chunk(b, u) for u in (1, 3, 5, 7)])

        # prologue: batch 0 first-half projections
        load_batch(0)
        for u in first_half(0):
            u()

        for b in range(BPC):
            fixed = second_half(b)          # 12 units -> pairs 0-3
            oproj = ([lambda oc=oc, bb=b - 1: outproj_chunk(bb, oc)
                      for oc in range(CH)] if b >= 1 else [])
            for o in range(CH):
                if o == 4 and b + 1 < BPC:
                    load_batch(b + 1)       # safe: batch-b x tiles released
                units = []
                if o < 4:
                    units += fixed[3 * o:3 * o + 3]
                elif b + 1 < BPC:
                    units += first_half(b + 1)[3 * (o - 4):3 * (o - 4) + 3]
                units += oproj[o::CH]
                it = iter(units)

                def pull(n):
                    for _ in range(n):
                        u = next(it, None)
                        if u is not None:
                            u()

                attn_pair(b, o, pull)
                pull(len(units))
            if b >= 1:
                del tiles[b - 1]
        for oc in range(CH):
            outproj_chunk(BPC - 1, oc)
        del tiles[BPC - 1]

    nc.compile()
    return nc


def _col(v):
    """[E] bias -> [128, CH] per-partition column layout (e = c*128 + p)."""
    return np.ascontiguousarray(v.reshape(CH, P).T.astype(np.float32))


def kernel(query, key, value, attn_mask, pe, Wq, bq, Wk, bk, Wv, bv, Wp, bp,
           Wo, bo, gates):
    bf16 = ml_dtypes.bfloat16
    f32 = np.float32
    query, key, value = (np.asarray(x, f32) for x in (query, key, value))
    attn_mask = np.asarray(attn_mask, f32)
    pe = np.asarray(pe, f32)
    Wq, bq, Wk, bk, Wv, bv, Wp, bp, Wo, bo, gates = (
        np.asarray(x, f32) for x in (Wq, bq, Wk, bk, Wv, bv, Wp, bp, Wo, bo,
                                     gates))

    use_mask = bool(np.any(attn_mask))
    key_nc = ("nc", use_mask)
    if key_nc not in _cache:
        _cache[key_nc] = _build(use_mask)
    nc = _cache[key_nc]

    def t_bf16(x):  # [.., A, B] -> [.., B, A] contiguous bf16
        return np.ascontiguousarray(np.swapaxes(x, -1, -2)).astype(bf16)

    common = {
        "wqT": t_bf16(Wq), "wkT": t_bf16(Wk), "wvT": t_bf16(Wv),
        "woT": t_bf16(Wo), "wpT": t_bf16(Wp),
        "peT": t_bf16(pe[0, :S]),
        "bpq": _col(bp + bq), "bpk": _col(bp + bk),
        "bvp": np.ascontiguousarray((bp + bv)[None, :]).astype(bf16),
        "bo_col": _col(bo),
        "gates11": gates.reshape(1, 1).astype(f32),
    }
    if use_mask:
        mT = np.zeros((SP, S), f32)
        mT[:S, :] = attn_mask.T
        common["maskT"] = mT.astype(bf16)

    qT = t_bf16(query)   # [B, E, S]
    kT = t_bf16(key)
    vT = t_bf16(value)

    in_maps = []
    for i in range(NC):
        sl = slice(i * BPC, (i + 1) * BPC)
        m = dict(common)
        m["xq"] = np.ascontiguousarray(qT[sl])
        m["xk"] = np.ascontiguousarray(kT[sl])
        m["xv"] = np.ascontiguousarray(vT[sl])
        in_maps.append(m)

    global _in_maps_cache
    _in_maps_cache = in_maps

    from concourse.bass_utils import run_bass_kernel_spmd
    res = run_bass_kernel_spmd(nc, in_maps, core_ids=list(range(NC)))

    out = np.empty((B, S, E), f32)
    for i in range(NC):
        outT = res.results[i]["outT"]          # [BPC, E, S]
        out[i * BPC:(i + 1) * BPC] = np.swapaxes(outT, 1, 2)
    return out


# revision 20
# speedup vs baseline: 1.0491x; 1.0185x over previous
"""Trainium2 Bass kernel for LowRankMultiheadAttention (B=32, S=400, E=1024, H=16).

Strategy: data-parallel over batch across 8 NeuronCores (4 batches/core).
Per core, activations live in "transposed space" (embed dim on partitions) so
every matmul contracts over the SBUF partition dim:

  posT   = WpT.T @ peT                      (positional projection, once)
  qT     = WqT.T @ xqT + (posT + bp + bq)   (per batch, [E, S] layout)
  kT     = WkT.T @ xkT + (posT + bp + bk)
  v_nat  = xvT.T @ WvT + (pos_nat+bp+bv)    (natural [S, E] layout, 65-stride
                                             per head with a ones column)
  per head pair (2 heads share a 128-partition chunk):
    scoresT    = kT_head.T @ qT_head        (row-packed, 2 heads concurrent)
    expT       = Exp(0.125 * scoresT)       (2 key-chunks fused per ACT op)
    attnv      = v_aug.T @ expT             (accum over 4 key chunks; row 64 =
                                             softmax denominator via ones col)
    out_head   = attnv[0:64] * bcast(approx_recip(attnv[64]))
  outT   = WoT.T @ attn_outT; final = gates*outT + gates*bo

Emission interleaves batch b's attention with batch b+1's projections so the
TensorE never idles long enough for the HAM clock-gate to re-throttle.
Host side does layout prep only (transpose/cast/shard + final gather).
"""

from contextlib import ExitStack

import numpy as np
import ml_dtypes

B, S, E, H, D = 32, 400, 1024, 16, 64
NC = 8
BPC = B // NC          # batches per core
SP = 512               # padded key length
CH = E // 128          # 128-partition chunks of the embed dim
SCALE = 1.0 / 8.0      # 1/sqrt(D)
P = 128

_cache: dict = {}
_in_maps_cache = None


def _build(use_mask: bool):
    import concourse.mybir as mybir
    import concourse.tile as tile
    from concourse import bacc

    F32 = mybir.dt.float32
    BF16 = mybir.dt.bfloat16
    AF = mybir.ActivationFunctionType

    nc = bacc.Bacc("TRN2", target_bir_lowering=False, debug=False,
                   enable_asserts=False)

    xq_d = nc.dram_tensor("xq", [BPC, E, S], BF16, kind="ExternalInput")
    xk_d = nc.dram_tensor("xk", [BPC, E, S], BF16, kind="ExternalInput")
    xv_d = nc.dram_tensor("xv", [BPC, E, S], BF16, kind="ExternalInput")
    wq_d = nc.dram_tensor("wqT", [E, E], BF16, kind="ExternalInput")
    wk_d = nc.dram_tensor("wkT", [E, E], BF16, kind="ExternalInput")
    wv_d = nc.dram_tensor("wvT", [E, E], BF16, kind="ExternalInput")
    wo_d = nc.dram_tensor("woT", [E, E], BF16, kind="ExternalInput")
    wp_d = nc.dram_tensor("wpT", [E, E], BF16, kind="ExternalInput")
    pe_d = nc.dram_tensor("peT", [E, S], BF16, kind="ExternalInput")
    bpq_d = nc.dram_tensor("bpq", [P, CH], F32, kind="ExternalInput")
    bpk_d = nc.dram_tensor("bpk", [P, CH], F32, kind="ExternalInput")
    bvp_d = nc.dram_tensor("bvp", [1, E], BF16, kind="ExternalInput")
    bo_d = nc.dram_tensor("bo_col", [P, CH], F32, kind="ExternalInput")
    g_d = nc.dram_tensor("gates11", [1, 1], F32, kind="ExternalInput")
    if use_mask:
        mask_d = nc.dram_tensor("maskT", [SP, S], BF16, kind="ExternalInput")
    out_d = nc.dram_tensor("outT", [BPC, E, S], F32, kind="ExternalOutput")

    def dram3(d):  # [E, x] row-major -> [128, CH, x] partition view
        return d.ap().rearrange("(c p) t -> p c t", p=P)

    with tile.TileContext(nc) as tc, ExitStack() as ctx:
        consts = ctx.enter_context(tc.tile_pool(name="consts", bufs=1))
        wpool = ctx.enter_context(tc.tile_pool(name="weights", bufs=1))

        PQ = consts.tile([P, CH, S], BF16, tag="PQ")
        PK = consts.tile([P, CH, S], BF16, tag="PK")
        PV = consts.tile([P, 4, E], BF16, tag="PV")
        bvpb = consts.tile([P, E], BF16, tag="bvpb")
        ones_bf = consts.tile([1, P], BF16, tag="ones_bf")
        ones_f32 = consts.tile([1, P], F32, tag="ones_f32")
        zeros_col = consts.tile([P, 1], F32, tag="zeros_col")
        gates_col = consts.tile([P, 1], F32, tag="gates_col")
        bo_g = consts.tile([P, CH], F32, tag="bo_g")
        bpq = consts.tile([P, CH], F32, tag="bpq")
        bpk = consts.tile([P, CH], F32, tag="bpk")
        bo_c = consts.tile([P, CH], F32, tag="bo_c")
        if use_mask:
            maskT = consts.tile([P, 4, S], BF16, tag="maskT")
            nc.sync.dma_start(maskT[:], mask_d.ap().rearrange("(c p) t -> p c t", p=P))

        nc.gpsimd.dma_start(bpq[:], bpq_d.ap())
        nc.gpsimd.dma_start(bpk[:], bpk_d.ap())
        nc.gpsimd.dma_start(bo_c[:], bo_d.ap())
        nc.vector.memset(ones_bf[:], 1.0)
        nc.vector.memset(ones_f32[:], 1.0)
        nc.vector.memset(zeros_col[:], 0.0)

        # ---------------- startup: positional projections + scalar prep ----
        with tc.tile_pool(name="startup", bufs=1) as spool, \
             tc.tile_pool(name="spsum", bufs=2, space="PSUM") as spsum:
            wpT = spool.tile([P, CH, E], BF16, tag="wpT")
            peT = spool.tile([P, CH, S], BF16, tag="peT")
            g_sb = spool.tile([1, 1], F32, tag="g_sb")
            bvp_sb = spool.tile([1, E], BF16, tag="bvp_sb")
            nc.sync.dma_start(peT[:], dram3(pe_d))
            nc.sync.dma_start(wpT[:, :, 0:512], dram3(wp_d)[:, :, 0:512])
            nc.sync.dma_start(wpT[:, :, 512:E], dram3(wp_d)[:, :, 512:E])
            nc.gpsimd.dma_start(g_sb[:], g_d.ap())
            nc.gpsimd.dma_start(bvp_sb[:], bvp_d.ap())

            # gates broadcast down partitions (K=1 fp32 matmul)
            gp = spsum.tile([P, 512], F32, tag="sp")
            nc.tensor.matmul(gp[:, 0:1], ones_f32[:], g_sb[:], start=True, stop=True)
            nc.scalar.copy(gates_col[:], gp[:, 0:1])
            nc.vector.tensor_scalar_mul(bo_g[:], bo_c[:], gates_col[:, 0:1])

            # (bp+bv) broadcast down partitions
            for hf in range(2):
                ps = spsum.tile([P, 512], F32, tag="sp")
                nc.tensor.matmul(ps[:], ones_bf[:], bvp_sb[0:1, hf * 512:(hf + 1) * 512],
                                 start=True, stop=True)
                nc.scalar.copy(bvpb[:, hf * 512:(hf + 1) * 512], ps[:])

            # posT -> PQ / PK  (transposed orientation)
            for ec in range(CH):
                ps = spsum.tile([P, 512], F32, tag="sp")
                for dc in range(CH):
                    nc.tensor.matmul(ps[:, 0:S], wpT[:, dc, ec * P:(ec + 1) * P],
                                     peT[:, dc, :], start=dc == 0, stop=dc == CH - 1)
                nc.scalar.activation(PQ[:, ec, :], ps[:, 0:S], AF.Identity,
                                     bias=bpq[:, ec:ec + 1])
                nc.scalar.activation(PK[:, ec, :], ps[:, 0:S], AF.Identity,
                                     bias=bpk[:, ec:ec + 1])

            # pos_nat -> PV  (natural orientation)
            for tt in range(4):
                tsz = P if tt < 3 else S - 3 * P
                for hf in range(2):
                    ps = spsum.tile([P, 512], F32, tag="sp")
                    for dc in range(CH):
                        nc.tensor.matmul(ps[0:tsz, :], peT[:, dc, tt * P:tt * P + tsz],
                                         wpT[:, dc, hf * 512:(hf + 1) * 512],
                                         start=dc == 0, stop=dc == CH - 1)
                    nc.vector.tensor_add(PV[0:tsz, tt, hf * 512:(hf + 1) * 512],
                                         ps[0:tsz, :],
                                         bvpb[0:tsz, hf * 512:(hf + 1) * 512])

        # ---------------- resident weights --------------------------------
        wq = wpool.tile([P, CH, E], BF16, tag="wq")
        wk = wpool.tile([P, CH, E], BF16, tag="wk")
        wv = wpool.tile([P, CH, E], BF16, tag="wv")
        wo = wpool.tile([P, CH, E], BF16, tag="wo")
        nc.gpsimd.dma_start(wq[:], dram3(wq_d))
        nc.gpsimd.dma_start(wk[:], dram3(wk_d))
        nc.gpsimd.dma_start(wv[:], dram3(wv_d))
        nc.gpsimd.dma_start(wo[:], dram3(wo_d))

        # ---------------- main pools ---------------------------------------
        xin = ctx.enter_context(tc.tile_pool(name="xin", bufs=3))
        qtp = ctx.enter_context(tc.tile_pool(name="qt", bufs=2))
        ktp = ctx.enter_context(tc.tile_pool(name="kt", bufs=2))
        vap = ctx.enter_context(tc.tile_pool(name="va", bufs=2))
        aop = ctx.enter_context(tc.tile_pool(name="ao", bufs=3))
        exq = ctx.enter_context(tc.tile_pool(name="ex", bufs=4))
        bcp = ctx.enter_context(tc.tile_pool(name="bc", bufs=3))
        rcp = ctx.enter_context(tc.tile_pool(name="rc", bufs=4))
        fnp = ctx.enter_context(tc.tile_pool(name="fn", bufs=3))
        mmp = ctx.enter_context(tc.tile_pool(name="mm", bufs=2, space="PSUM"))
        scp = ctx.enter_context(tc.tile_pool(name="sc", bufs=2, space="PSUM"))
        avp = ctx.enter_context(tc.tile_pool(name="av", bufs=2, space="PSUM"))

        tiles = {}

        def load_batch(b):
            t = {}
            t["xq"] = xin.tile([P, CH, S], BF16, tag="xin", name=f"xq{b}")
            nc.sync.dma_start(t["xq"][:], xq_d.ap()[b].rearrange("(c p) t -> p c t", p=P))
            t["xk"] = xin.tile([P, CH, S], BF16, tag="xin", name=f"xk{b}")
            nc.sync.dma_start(t["xk"][:], xk_d.ap()[b].rearrange("(c p) t -> p c t", p=P))
            t["xv"] = xin.tile([P, CH, S], BF16, tag="xin", name=f"xv{b}")
            nc.sync.dma_start(t["xv"][:], xv_d.ap()[b].rearrange("(c p) t -> p c t", p=P))
            t["qT"] = qtp.tile([P, CH, S], BF16, tag="qt", name=f"qT{b}")
            t["kT"] = ktp.tile([P, CH, SP], BF16, tag="kt", name=f"kT{b}")
            nc.vector.memset(t["kT"][:, :, S:SP], 0.0)
            va = vap.tile([P, 4, H, D + 1], BF16, tag="va", name=f"va{b}")
            nc.vector.memset(va[:, :, :, D], 1.0)   # ones column
            nc.vector.memset(va[:, 3, :, :], 0.0)   # zero padded key rows
            nc.vector.memset(va[0:S - 3 * P, 3, :, D], 1.0)  # restore valid ones
            t["va"] = va
            t["ao"] = aop.tile([P, CH, S], BF16, tag="ao", name=f"ao{b}")
            tiles[b] = t

        def proj_q_chunk(b, ec):
            t = tiles[b]
            ps = mmp.tile([P, 512], F32, tag="mm", name=f"psq{b}_{ec}")
            for dc in range(CH):
                nc.tensor.matmul(ps[:, 0:S], wq[:, dc, ec * P:(ec + 1) * P],
                                 t["xq"][:, dc, :], start=dc == 0, stop=dc == CH - 1)
            nc.vector.tensor_add(t["qT"][:, ec, :], ps[:, 0:S], PQ[:, ec, :])

        def proj_k_chunk(b, ec):
            t = tiles[b]
            ps = mmp.tile([P, 512], F32, tag="mm", name=f"psk{b}_{ec}")
            for dc in range(CH):
                nc.tensor.matmul(ps[:, 0:S], wk[:, dc, ec * P:(ec + 1) * P],
                                 t["xk"][:, dc, :], start=dc == 0, stop=dc == CH - 1)
            nc.vector.tensor_add(t["kT"][:, ec, 0:S], ps[:, 0:S], PK[:, ec, :])

        def proj_v_chunk(b, u):
            t = tiles[b]
            tt, hf = u // 2, u % 2
            tsz = P if tt < 3 else S - 3 * P
            ps = mmp.tile([P, 512], F32, tag="mm", name=f"psv{b}_{u}")
            for dc in range(CH):
                nc.tensor.matmul(ps[0:tsz, :], t["xv"][:, dc, tt * P:tt * P + tsz],
                                 wv[:, dc, hf * 512:(hf + 1) * 512],
                                 start=dc == 0, stop=dc == CH - 1)
            nc.vector.tensor_add(
                t["va"][0:tsz, tt, hf * 8:(hf + 1) * 8, 0:D],
                ps[0:tsz, :].rearrange("p (h d) -> p h d", d=D),
                PV[0:tsz, tt, hf * 512:(hf + 1) * 512].rearrange(
                    "p (h d) -> p h d", d=D))

        def attn_pair(b, o, pull=None):
            t = tiles[b]
            qT, kT, va, ao = t["qT"], t["kT"], t["va"], t["ao"]
            av0 = avp.tile([D + 1, S], F32, tag="av", name=f"av0_{b}_{o}")
            av1 = avp.tile([D + 1, S], F32, tag="av", name=f"av1_{b}_{o}")
            for cc in range(2):
                sA = scp.tile([P, 2, 512], F32, tag="sc", name=f"sA{b}_{o}_{cc}")
                sB = scp.tile([P, 2, 512], F32, tag="sc", name=f"sB{b}_{o}_{cc}")
                for i in range(2):
                    c = 2 * cc + i
                    nc.tensor.matmul(sA[:, i, 0:S], kT[0:D, o, c * P:(c + 1) * P],
                                     qT[0:D, o, :], start=True, stop=True,
                                     tile_position=(0, 0))
                    nc.tensor.matmul(sB[:, i, 0:S], kT[D:P, o, c * P:(c + 1) * P],
                                     qT[D:P, o, :], start=True, stop=True,
                                     tile_position=(64, 0))
                    if use_mask:
                        nc.vector.tensor_add(sA[:, i, 0:S], sA[:, i, 0:S],
                                             maskT[:, c, :])
                        nc.vector.tensor_add(sB[:, i, 0:S], sB[:, i, 0:S],
                                             maskT[:, c, :])
                eA = exq.tile([P, 2, S], BF16, tag="ex", name=f"eA{b}_{o}_{cc}")
                eB = exq.tile([P, 2, S], BF16, tag="ex", name=f"eB{b}_{o}_{cc}")
                nc.scalar.activation(eA[:], sA[:, :, 0:S], AF.Exp, bias=zeros_col[:],
                                     scale=SCALE)
                nc.scalar.activation(eB[:], sB[:, :, 0:S], AF.Exp, bias=zeros_col[:],
                                     scale=SCALE)
                for i in range(2):
                    c = 2 * cc + i
                    nc.tensor.matmul(av0[:], va[:, c, 2 * o, :], eA[:, i, :],
                                     start=c == 0, stop=c == 3)
                    nc.tensor.matmul(av1[:], va[:, c, 2 * o + 1, :], eB[:, i, :],
                                     start=c == 0, stop=c == 3)
                if pull is not None:
                    pull(1)
            d0 = rcp.tile([1, S], F32, tag="den", name=f"d0{b}_{o}")
            d1 = rcp.tile([1, S], F32, tag="den", name=f"d1{b}_{o}")
            nc.vector.tensor_copy(d0[:], av0[D:D + 1, :])
            nc.vector.tensor_copy(d1[:], av1[D:D + 1, :])
            r0f = rcp.tile([1, S], F32, tag="rcf", name=f"r0f{b}_{o}")
            r1f = rcp.tile([1, S], F32, tag="rcf", name=f"r1f{b}_{o}")
            nc.vector.reciprocal_approx_fast(r0f[:], d0[:])
            nc.vector.reciprocal_approx_fast(r1f[:], d1[:])
            bc0 = bcp.tile([D, S], F32, tag="bc0", name=f"bc0{b}_{o}")
            bc1 = bcp.tile([D, S], F32, tag="bc1", name=f"bc1{b}_{o}")
            nc.gpsimd.partition_broadcast(bc0[:], r0f[:])
            nc.gpsimd.partition_broadcast(bc1[:], r1f[:])
            nc.vector.tensor_mul(ao[0:D, o, :], av0[0:D, :], bc0[:])
            nc.vector.tensor_mul(ao[D:P, o, :], av1[0:D, :], bc1[:])

        def outproj_chunk(b, oc):
            t = tiles[b]
            ps = mmp.tile([P, 512], F32, tag="mm", name=f"pso{b}_{oc}")
            for ec in range(CH):
                nc.tensor.matmul(ps[:, 0:S], wo[:, ec, oc * P:(oc + 1) * P],
                                 t["ao"][:, ec, :], start=ec == 0,
                                 stop=ec == CH - 1)
            fn = fnp.tile([P, S], F32, tag="fn", name=f"fn{b}_{oc}")
            nc.scalar.activation(fn[:], ps[:, 0:S], AF.Identity,
                                 bias=bo_g[:, oc:oc + 1],
                                 scale=gates_col[:, 0:1])
            nc.sync.dma_start(
                out_d.ap()[b].rearrange("(c p) t -> p c t", p=P)[:, oc, :],
                fn[:])

        def first_half(b):
            # chunks needed by attention pairs 0-3: q/k ec 0-3, v heads 0-7
            return ([lambda ec=ec: proj_q_chunk(b, ec) for ec in range(4)]
                    + [lambda ec=ec: proj_k_chunk(b, ec) for ec in range(4)]
                    + [lambda u=u: proj_v_chunk(b, u) for u in (0, 2, 4, 6)])

        def second_half(b):
            # chunks first needed at pair 4: woven into pairs 0-3 of attn(b)
            return ([lambda ec=ec: proj_q_chunk(b, ec) for ec in range(4, CH)]
                    + [lambda ec=ec: proj_k_chunk(b, ec) for ec in range(4, CH)]
                    + [lambda u=u: proj_v_chunk(b, u) for u in (1, 3, 5, 7)])

        # prologue: batch 0 first-half projections
        load_batch(0)
        for u in first_half(0):
            u()

        for b in range(BPC):
            fixed = second_half(b)          # 12 units -> pairs 0-3
            oproj = ([lambda oc=oc, bb=b - 1: outproj_chunk(bb, oc)
                      for oc in range(CH)] if b >= 1 else [])
            for o in range(CH):
                if o == 4 and b + 1 < BPC:
                    load_batch(b + 1)       # safe: batch-b x tiles released
                units = []
                if o < 4:
                    units += fixed[3 * o:3 * o + 3]
                elif b + 1 < BPC:
                    units += first_half(b + 1)[3 * (o - 4):3 * (o - 4) + 3]
                units += oproj[o::CH]
                it = iter(units)

                def pull(n):
                    for _ in range(n):
                        u = next(it, None)
                        if u is not None:
                            u()

                attn_pair(b, o, pull)
                pull(len(units))
            if b >= 1:
                del tiles[b - 1]
        for oc in range(CH):
            outproj_chunk(BPC - 1, oc)
        del tiles[BPC - 1]

    nc.compile()
    return nc


def _col(v):
    """[E] bias -> [128, CH] per-partition column layout (e = c*128 + p)."""
    return np.ascontiguousarray(v.reshape(CH, P).T.astype(np.float32))


def kernel(query, key, value, attn_mask, pe, Wq, bq, Wk, bk, Wv, bv, Wp, bp,
           Wo, bo, gates):
    bf16 = ml_dtypes.bfloat16
    f32 = np.float32
    query, key, value = (np.asarray(x, f32) for x in (query, key, value))
    attn_mask = np.asarray(attn_mask, f32)
    pe = np.asarray(pe, f32)
    Wq, bq, Wk, bk, Wv, bv, Wp, bp, Wo, bo, gates = (
        np.asarray(x, f32) for x in (Wq, bq, Wk, bk, Wv, bv, Wp, bp, Wo, bo,
                                     gates))

    use_mask = bool(np.any(attn_mask))
    key_nc = ("nc", use_mask)
    if key_nc not in _cache:
        _cache[key_nc] = _build(use_mask)
    nc = _cache[key_nc]

    def t_bf16(x):  # [.., A, B] -> [.., B, A] contiguous bf16
        return np.ascontiguousarray(np.swapaxes(x, -1, -2)).astype(bf16)

    common = {
        "wqT": t_bf16(Wq), "wkT": t_bf16(Wk), "wvT": t_bf16(Wv),
        "woT": t_bf16(Wo), "wpT": t_bf16(Wp),
        "peT": t_bf16(pe[0, :S]),
        "bpq": _col(bp + bq), "bpk": _col(bp + bk),
        "bvp": np.ascontiguousarray((bp + bv)[None, :]).astype(bf16),
        "bo_col": _col(bo),
        "gates11": gates.reshape(1, 1).astype(f32),
    }
    if use_mask:
        mT = np.zeros((SP, S), f32)
        mT[:S, :] = attn_mask.T
        common["maskT"] = mT.astype(bf16)

    qT = t_bf16(query)   # [B, E, S]
    kT = t_bf16(key)
    vT = t_bf16(value)

    in_maps = []
    for i in range(NC):
        sl = slice(i * BPC, (i + 1) * BPC)
        m = dict(common)
        m["xq"] = np.ascontiguousarray(qT[sl])
        m["xk"] = np.ascontiguousarray(kT[sl])
        m["xv"] = np.ascontiguousarray(vT[sl])
        in_maps.append(m)

    global _in_maps_cache
    _in_maps_cache = in_maps

    from concourse.bass_utils import run_bass_kernel_spmd
    res = run_bass_kernel_spmd(nc, in_maps, core_ids=list(range(NC)))

    out = np.empty((B, S, E), f32)
    for i in range(NC):
        outT = res.results[i]["outT"]          # [BPC, E, S]
        out[i * BPC:(i + 1) * BPC] = np.swapaxes(outT, 1, 2)
    return out
